# revision 26
# baseline (speedup 1.0000x reference)
"""Trainium2 Bass kernel: grouped-experts SwiGLU MLP with mid-RMSNorm.

Expert-parallel across 8 NeuronCores: core e computes expert e's token
block (tokens are pre-sorted by expert).

v2: weight-stationary ("flipped") matmul orientation.  The moving
operand is the token axis, so PE cost scales with the actual padded
token count (576, vs 640 tile-padded in v1) and the SwiGLU hidden state
is produced directly in [f, tok] layout -- exactly what the down-proj
matmul consumes -- eliminating all PE transposes.  The mid-RMSNorm
row scale commutes with the (linear) down projection, so the device
only produces ssq[tok] = sum_f h^2 (via a ones-column matmul) and the
host applies rstd; mid_w is folded into w2.

Per-core math (fp16 in / fp32 accumulate), NTOK = 576 padded tokens:
    h1[f,t] = sum_dk w1c[dk].T @ x[dk]     (16 accum MMs per f-chunk)
    h  = silu(h1) * h3                     # [128, KF, NTOK] in SBUF
    ssq[t] = ones.T @ (h*h)                # [1, NTOK] accum over KF
    out[d,t] = sum_fk w2c[fk].T @ h[fk]    # [KD, 128, NTOK]
Host: out_rows *= rsqrt(ssq/F + eps); scatter to flat token order.

PSUM: one pool, 4 tags x full 2KB bank x 2 bufs = 8 banks exactly; no
two accumulation tiles share a bank, so PE-write never collides with
ScalarE/VectorE reads of a neighbouring tile.

DMA: w1 on sync, w3 on gpsimd, x + w2 on scalar (w2 is only needed for
phase C at ~2/3 of the kernel), outputs alternate sync/gpsimd.  First
slabs of x and w1/w3 are split small so the first matmuls gate early;
a few warm-up matmuls plus mid-fk0 filler matmuls keep the PE activity
window busy through the DMA-bound startup so the HAM clock gate does
not re-throttle mid-phase-A.
"""

import sys

sys.path.insert(0, "/opt/trn_rl_repo")

import os

import numpy as np
from contextlib import ExitStack

import concourse.bass as bass
import concourse.tile as tile
from concourse import bacc, mybir

P = 128
D = 2048
F = 1024
E = 8
KD = D // P  # 16 contraction chunks for mm1/mm3
KF = F // P  # 8 f chunks (contraction chunks for mm2)
EPS = 1e-6
F32 = mybir.dt.float32
F16 = mybir.dt.float16
ACTF = mybir.ActivationFunctionType

_PROGRAM_CACHE: dict[int, object] = {}
LAST_RESULTS = None  # test harness reads per-core outputs from here


def _run(nc, in_maps):
    """Execute the compiled program on the 8 axon-tunneled cores.

    If KERNEL_NTFF_DIR is set, wrap the execute in the axon NTFF profile
    hook so device profiles land there (test harness use only).
    """
    from concourse import bass2jax

    ntff_dir = os.environ.get("KERNEL_NTFF_DIR")
    if ntff_dir:
        if "/root/.axon_site" not in sys.path:
            sys.path.insert(0, "/root/.axon_site")
        from trn_agent_boot.trn_boot import _ntff_profile_via_ctypes

        hook = _ntff_profile_via_ctypes("/opt/axon/libaxon_pjrt.so")
        ids = [
            int(x) for x in os.environ.get("KERNEL_NTFF_CORES", "0").split(",")
        ]
        if hook is not None:
            with hook(ntff_dir, ids):
                return bass2jax.run_bass_via_pjrt(nc, in_maps, n_cores=len(in_maps))
    return bass2jax.run_bass_via_pjrt(nc, in_maps, n_cores=len(in_maps))


def _build_program(NTOK: int):
    """Build + compile the single-core SPMD program for NTOK padded tokens."""
    # token chunks, each <= 512 (one fp32 PSUM bank of moving dim)
    CH = [(0, min(512, NTOK))]
    if NTOK > 512:
        CH.append((512, NTOK))
    NCH = len(CH)

    nc = bacc.Bacc(
        "TRN2",
        target_bir_lowering=False,
        debug=False,
        enable_asserts=False,
        num_devices=E,
    )
    xT_d = nc.dram_tensor("xT", [P, KD, NTOK], F16, kind="ExternalInput").ap()
    w1_d = nc.dram_tensor("w1t", [P, KF, KD, P], F16, kind="ExternalInput").ap()
    w3_d = nc.dram_tensor("w3t", [P, KF, KD, P], F16, kind="ExternalInput").ap()
    w2_d = nc.dram_tensor("w2t", [P, KD, KF, P], F16, kind="ExternalInput").ap()
    out_d = nc.dram_tensor("out", [P, KD, NTOK], F16, kind="ExternalOutput").ap()
    ssq_d = nc.dram_tensor("ssq", [1, NTOK], F32, kind="ExternalOutput").ap()

    with tile.TileContext(nc) as tc, ExitStack() as ctx:
        singles = ctx.enter_context(tc.tile_pool(name="singles", bufs=1))
        xpool = ctx.enter_context(tc.tile_pool(name="x", bufs=1))
        w1pool = ctx.enter_context(tc.tile_pool(name="w1", bufs=1))
        w3pool = ctx.enter_context(tc.tile_pool(name="w3", bufs=1))
        w2pool = ctx.enter_context(tc.tile_pool(name="w2", bufs=1))
        hpool = ctx.enter_context(tc.tile_pool(name="h", bufs=1))
        qpool = ctx.enter_context(tc.tile_pool(name="hsq", bufs=1))
        spool = ctx.enter_context(tc.tile_pool(name="scr", bufs=2))
        opool = ctx.enter_context(tc.tile_pool(name="o", bufs=1))
        psp = ctx.enter_context(tc.tile_pool(name="ps", bufs=2, space="PSUM"))

        warm = singles.tile([P, 512], F16, name="warm")
        nc.gpsimd.memset(warm[:], 0.5)
        ones = singles.tile([P, 1], F16, name="ones")
        nc.gpsimd.memset(ones[:], 1.0)
        ssq_sb = singles.tile([1, NTOK], F32, name="ssq_sb")

        xsb = xpool.tile([P, KD, NTOK], F16)
        w1sb = w1pool.tile([P, KF, KD, P], F16)
        w3sb = w3pool.tile([P, KF, KD, P], F16)
        w2sb = w2pool.tile([P, KD, KF, P], F16)
        h = hpool.tile([P, KF, NTOK], F16)
        hsq = qpool.tile([P, KF, NTOK], F16)

        # ---- DMA schedule (consumption order; 3 parallel queues) --------
        # Startup-critical bytes: all of x (re-read per fk from SBUF, so the
        # full 2.4MB gates fk0's last dk chunks) + fk0 weights.  Spread x
        # over all three queues; w2 rides sync/gpsimd AFTER w1/w3 (never on
        # scalar: the ACT queue is FIFO and must stay free for phase-A
        # sigmoids, which release PSUM).
        nc.scalar.dma_start(xsb[:, 0:1, :], xT_d[:, 0:1, :])
        nc.sync.dma_start(w1sb[:, 0, 0:2, :], w1_d[:, 0, 0:2, :])
        nc.gpsimd.dma_start(w3sb[:, 0, 0:2, :], w3_d[:, 0, 0:2, :])
        nc.scalar.dma_start(xsb[:, 1:4, :], xT_d[:, 1:4, :])
        nc.sync.dma_start(w1sb[:, 0, 2:4, :], w1_d[:, 0, 2:4, :])
        nc.gpsimd.dma_start(w3sb[:, 0, 2:4, :], w3_d[:, 0, 2:4, :])
        nc.sync.dma_start(w1sb[:, 0, 4:KD, :], w1_d[:, 0, 4:KD, :])
        nc.gpsimd.dma_start(w3sb[:, 0, 4:KD, :], w3_d[:, 0, 4:KD, :])
        nc.scalar.dma_start(xsb[:, 4:6, :], xT_d[:, 4:6, :])
        nc.sync.dma_start(xsb[:, 6:10, :], xT_d[:, 6:10, :])
        nc.gpsimd.dma_start(xsb[:, 10:13, :], xT_d[:, 10:13, :])
        nc.gpsimd.dma_start(xsb[:, 13:KD, :], xT_d[:, 13:KD, :])
        for fk in range(1, KF):
            nc.sync.dma_start(w1sb[:, fk], w1_d[:, fk])
            nc.gpsimd.dma_start(w3sb[:, fk], w3_d[:, fk])
        nc.sync.dma_start(w2sb[:, 0:8], w2_d[:, 0:8])
        nc.gpsimd.dma_start(w2sb[:, 8:KD], w2_d[:, 8:KD])

        # HAM warm-up: keep the PE busy through the prologue DMA wait so
        # the clock gate is at 8/8 when the first real matmul's data lands.
        PSW = 1024 if NTOK > 512 else 512  # psum tile: 2 banks if tail chunk
        ps_w = psp.tile([P, PSW], F32, tag="pa", name="warm_ps")
        for _ in range(5):
            nc.tensor.matmul(ps_w[:, 0:512], warm[:, 0:P], warm[:], start=True, stop=True)

        # ================= phase A: h = silu(x@w1^T) * (x@w3^T) ===========
        # One [P, PSW] psum tile per h1/h3: the 512-wide chunk lands in the
        # first bank, the tail in the second, so the epilogue runs as single
        # wide ops over [0:NTOK] (fewer instructions + semaphore edges).
        for fk in range(KF):
            p1 = psp.tile([P, PSW], F32, tag="pa", name=f"p1_{fk}")
            p3 = psp.tile([P, PSW], F32, tag="pc", name=f"p3_{fk}")
            for dk in range(KD):
                if fk == 0 and dk in (4, 6):
                    # HAM keep-alive during the startup DMA stall: a few
                    # matmuls into the idle warm-up buffer bridge the gap so
                    # the clock gate never re-throttles to 4/8 mid-phase-A.
                    for _ in range(4):
                        nc.tensor.matmul(
                            ps_w[:, 0:512], warm[:, 0:P], warm[:], start=True, stop=True
                        )
                wc1 = w1sb[:, fk, dk, :]
                wc3 = w3sb[:, fk, dk, :]
                st = dk == 0
                sp = dk == KD - 1
                for a, b in CH:
                    nc.tensor.matmul(p1[:, a:b], wc1, xsb[:, dk, a:b], start=st, stop=sp)
                for a, b in CH:
                    nc.tensor.matmul(p3[:, a:b], wc3, xsb[:, dk, a:b], start=st, stop=sp)
            s = spool.tile([P, NTOK], F32, tag="sig", name=f"sig_{fk}")
            nc.scalar.activation(s[:], p1[:, 0:NTOK], ACTF.Sigmoid)
            hs = h[:, fk, :]
            nc.vector.tensor_mul(hs, s[:], p1[:, 0:NTOK])
            nc.vector.tensor_mul(hs, hs, p3[:, 0:NTOK])
            nc.vector.tensor_mul(hsq[:, fk, :], hs, hs)

        # ================= ssq[t] = sum_f h^2 (ones-column matmuls) =======
        sacc = psp.tile([P, PSW], F32, tag="pc", name="sacc")
        for fk in range(KF):
            for a, b in CH:
                nc.tensor.matmul(
                    sacc[0:1, a:b],
                    ones[:],
                    hsq[:, fk, a:b],
                    start=(fk == 0),
                    stop=(fk == KF - 1),
                )
        nc.vector.tensor_copy(ssq_sb[:], sacc[0:1, 0:NTOK])
        nc.scalar.dma_start(ssq_d[:, :], ssq_sb[:])

        # ================= phase C: out[d,t] = sum_fk w2c.T @ h ===========
        # output batches: big batches overlap phase C; tiny last batch so the
        # kernel end is not gated on a large transfer + completion receipt.
        OBATCH = [(0, 5), (5, 10), (10, 13), (13, 15), (15, 16)]
        obt = {
            g0: opool.tile([P, g1 - g0, NTOK], F16, tag=f"ob{g0}", name=f"ob{g0}")
            for g0, g1 in OBATCH
        }
        for gi, (g0, g1) in enumerate(OBATCH):
            ob = obt[g0]
            for dk in range(g0, g1):
                po = psp.tile([P, PSW], F32, tag="pa" if dk % 2 == 0 else "pc", name=f"po_{dk}")
                for fk in range(KF):
                    wc2 = w2sb[:, dk, fk, :]
                    st = fk == 0
                    sp = fk == KF - 1
                    for a, b in CH:
                        nc.tensor.matmul(po[:, a:b], wc2, h[:, fk, a:b], start=st, stop=sp)
                if g1 - g0 == 1 and g1 == KD:
                    # last chunk: split copy+DMA into halves on two queues so
                    # the two completion receipts overlap after the last MM.
                    hn = 288
                    nc.vector.tensor_copy(ob[:, 0, 0:hn], po[:, 0:hn])
                    nc.scalar.dma_start(out_d[:, g0:g1, 0:hn], ob[:, :, 0:hn])
                    nc.vector.tensor_copy(ob[:, 0, hn:NTOK], po[:, hn:NTOK])
                    nc.sync.dma_start(out_d[:, g0:g1, hn:NTOK], ob[:, :, hn:NTOK])
                else:
                    nc.vector.tensor_copy(ob[:, dk - g0, :], po[:, 0:NTOK])
            if not (g1 - g0 == 1 and g1 == KD):
                oq = nc.scalar if gi % 2 == 0 else nc.sync
                oq.dma_start(out_d[:, g0:g1, :], ob[:])

    nc.compile()
    return nc


def _get_program(NTOK: int):
    if NTOK not in _PROGRAM_CACHE:
        _PROGRAM_CACHE[NTOK] = _build_program(NTOK)
    return _PROGRAM_CACHE[NTOK]


def kernel(x, w1, w2, w3, mid_w, num_tokens_per_expert):
    global LAST_RESULTS
    x = np.ascontiguousarray(np.asarray(x, dtype=np.float32))
    w1 = np.asarray(w1, dtype=np.float32)
    w2 = np.asarray(w2, dtype=np.float32)
    w3 = np.asarray(w3, dtype=np.float32)
    mid_w = np.asarray(mid_w, dtype=np.float32)
    counts = np.asarray(num_tokens_per_expert).astype(np.int64)

    T_, D_ = x.shape
    E_, F_, _ = w1.shape
    Ccap = (T_ // E_) * 3 // 2  # reference static capacity (768)
    ends = np.cumsum(counts)
    starts = ends - counts
    eff = np.minimum(np.maximum(counts, 0), Ccap)  # rows actually computed

    NTOK = int(max(64, -(-int(eff.max()) // 64) * 64))  # pad to 64 tokens
    nc = _get_program(NTOK)

    in_maps = []
    for e in range(E_):
        cnt = int(eff[e])
        s = int(starts[e])
        xg = np.zeros((NTOK, D_), np.float32)
        if cnt > 0:
            rows = np.clip(s + np.arange(cnt), 0, T_ - 1)
            xg[:cnt] = x[rows]
        # xT: [P(p), KD, NTOK] with [p, dk, t] = x[t, dk*128+p]
        xT = xg.T.astype(np.float16).reshape(KD, P, NTOK).transpose(1, 0, 2)
        # w1t/w3t: [p, fk, dk, q] = w[fk*128+q, dk*128+p]   (w is [F, D])
        w1t = w1[e].astype(np.float16).reshape(KF, P, KD, P).transpose(3, 0, 2, 1)
        w3t = w3[e].astype(np.float16).reshape(KF, P, KD, P).transpose(3, 0, 2, 1)
        # w2t: [p, dk, fk, q] = w2m[dk*128+q, fk*128+p]     (w2m is [D, F])
        w2m = w2[e] * mid_w[None, :]
        w2t = w2m.astype(np.float16).reshape(KD, P, KF, P).transpose(3, 0, 2, 1)
        in_maps.append(
            {
                "xT": np.ascontiguousarray(xT),
                "w1t": np.ascontiguousarray(w1t),
                "w3t": np.ascontiguousarray(w3t),
                "w2t": np.ascontiguousarray(w2t),
            }
        )

    LAST_RESULTS = _run(nc, in_maps)

    outs = []
    for e in range(E_):
        o = (
            np.asarray(LAST_RESULTS[e]["out"], np.float32)
            .transpose(1, 0, 2)
            .reshape(D_, NTOK)
            .T
        )
        ssq = np.asarray(LAST_RESULTS[e]["ssq"], np.float32).reshape(NTOK)
        rstd = 1.0 / np.sqrt(ssq / F_ + EPS)
        outs.append(o * rstd[:, None])  # [NTOK, D]
    stacked = np.stack(outs, axis=0)  # [E, NTOK, D]

    # scatter back to flat token order, mirroring the reference's clamping
    tok = np.arange(T_)
    eid = np.clip(np.searchsorted(ends, tok, side="right"), 0, E_ - 1)
    pos = tok - starts[eid]
    idx = np.minimum(pos, Ccap - 1)
    valid = (idx >= 0) & (idx < eff[eid])
    idx_safe = np.clip(idx, 0, NTOK - 1)
    result = stacked[eid, idx_safe].astype(np.float32)
    result[~valid] = 0.0
    return result


# revision 27
# speedup vs baseline: 1.0453x; 1.0453x over previous
"""Trainium2 Bass kernel: grouped-experts SwiGLU MLP with mid-RMSNorm.

Expert-parallel across 8 NeuronCores: core e computes expert e's token
block (tokens are pre-sorted by expert).

v2: weight-stationary ("flipped") matmul orientation.  The moving
operand is the token axis, so PE cost scales with the actual padded
token count (576, vs 640 tile-padded in v1) and the SwiGLU hidden state
is produced directly in [f, tok] layout -- exactly what the down-proj
matmul consumes -- eliminating all PE transposes.  The mid-RMSNorm
row scale commutes with the (linear) down projection, so the device
only produces ssq[tok] = sum_f h^2 (via a ones-column matmul) and the
host applies rstd; mid_w is folded into w2.

Per-core math (fp16 in / fp32 accumulate), NTOK = 576 padded tokens:
    h1[f,t] = sum_dk w1c[dk].T @ x[dk]     (16 accum MMs per f-chunk)
    h  = silu(h1) * h3                     # [128, KF, NTOK] in SBUF
    ssq[t] = ones.T @ (h*h)                # [1, NTOK] accum over KF
    out[d,t] = sum_fk w2c[fk].T @ h[fk]    # [KD, 128, NTOK]
Host: out_rows *= rsqrt(ssq/F + eps); scatter to flat token order.

PSUM: one pool, 4 tags x full 2KB bank x 2 bufs = 8 banks exactly; no
two accumulation tiles share a bank, so PE-write never collides with
ScalarE/VectorE reads of a neighbouring tile.

DMA: w1 on sync, w3 on gpsimd, x + w2 on scalar (w2 is only needed for
phase C at ~2/3 of the kernel), outputs alternate sync/gpsimd.  First
slabs of x and w1/w3 are split small so the first matmuls gate early;
a few warm-up matmuls plus mid-fk0 filler matmuls keep the PE activity
window busy through the DMA-bound startup so the HAM clock gate does
not re-throttle mid-phase-A.
"""

import sys

sys.path.insert(0, "/opt/trn_rl_repo")

import os

import numpy as np
from contextlib import ExitStack

import concourse.bass as bass
import concourse.tile as tile
from concourse import bacc, mybir

P = 128
D = 2048
F = 1024
E = 8
KD = D // P  # 16 contraction chunks for mm1/mm3
KF = F // P  # 8 f chunks (contraction chunks for mm2)
EPS = 1e-6
F32 = mybir.dt.float32
F16 = mybir.dt.float16
ACTF = mybir.ActivationFunctionType

_PROGRAM_CACHE: dict[int, object] = {}
LAST_RESULTS = None  # test harness reads per-core outputs from here


def _run(nc, in_maps):
    """Execute the compiled program on the 8 axon-tunneled cores.

    If KERNEL_NTFF_DIR is set, wrap the execute in the axon NTFF profile
    hook so device profiles land there (test harness use only).
    """
    from concourse import bass2jax

    ntff_dir = os.environ.get("KERNEL_NTFF_DIR")
    if ntff_dir:
        if "/root/.axon_site" not in sys.path:
            sys.path.insert(0, "/root/.axon_site")
        from trn_agent_boot.trn_boot import _ntff_profile_via_ctypes

        hook = _ntff_profile_via_ctypes("/opt/axon/libaxon_pjrt.so")
        ids = [
            int(x) for x in os.environ.get("KERNEL_NTFF_CORES", "0").split(",")
        ]
        if hook is not None:
            with hook(ntff_dir, ids):
                return bass2jax.run_bass_via_pjrt(nc, in_maps, n_cores=len(in_maps))
    return bass2jax.run_bass_via_pjrt(nc, in_maps, n_cores=len(in_maps))


def _build_program(NTOK: int):
    """Build + compile the single-core SPMD program for NTOK padded tokens."""
    # token chunks, each <= 512 (one fp32 PSUM bank of moving dim)
    CH = [(0, min(512, NTOK))]
    if NTOK > 512:
        CH.append((512, NTOK))
    NCH = len(CH)

    nc = bacc.Bacc(
        "TRN2",
        target_bir_lowering=False,
        debug=False,
        enable_asserts=False,
        num_devices=E,
    )
    xT_d = nc.dram_tensor("xT", [P, KD, NTOK], F16, kind="ExternalInput").ap()
    w1_d = nc.dram_tensor("w1t", [P, KF, KD, P], F16, kind="ExternalInput").ap()
    w3_d = nc.dram_tensor("w3t", [P, KF, KD, P], F16, kind="ExternalInput").ap()
    w2_d = nc.dram_tensor("w2t", [P, KD, KF, P], F16, kind="ExternalInput").ap()
    out_d = nc.dram_tensor("out", [P, KD, NTOK], F16, kind="ExternalOutput").ap()
    ssq_d = nc.dram_tensor("ssq", [1, NTOK], F32, kind="ExternalOutput").ap()

    with tile.TileContext(nc) as tc, ExitStack() as ctx:
        singles = ctx.enter_context(tc.tile_pool(name="singles", bufs=1))
        xpool = ctx.enter_context(tc.tile_pool(name="x", bufs=1))
        w1pool = ctx.enter_context(tc.tile_pool(name="w1", bufs=1))
        w3pool = ctx.enter_context(tc.tile_pool(name="w3", bufs=1))
        w2pool = ctx.enter_context(tc.tile_pool(name="w2", bufs=1))
        hpool = ctx.enter_context(tc.tile_pool(name="h", bufs=1))
        qpool = ctx.enter_context(tc.tile_pool(name="hsq", bufs=1))
        spool = ctx.enter_context(tc.tile_pool(name="scr", bufs=2))
        opool = ctx.enter_context(tc.tile_pool(name="o", bufs=1))
        psp = ctx.enter_context(tc.tile_pool(name="ps", bufs=2, space="PSUM"))

        warm = singles.tile([P, 512], F16, name="warm")
        nc.gpsimd.memset(warm[:], 0.5)
        ones = singles.tile([P, 1], F16, name="ones")
        nc.gpsimd.memset(ones[:], 1.0)
        ssq_sb = singles.tile([1, NTOK], F32, name="ssq_sb")

        xsb = xpool.tile([P, KD, NTOK], F16)
        w1sb = w1pool.tile([P, KF, KD, P], F16)
        w3sb = w3pool.tile([P, KF, KD, P], F16)
        w2sb = w2pool.tile([P, KD, KF, P], F16)
        h = hpool.tile([P, KF, NTOK], F16)
        hsq = qpool.tile([P, KF, NTOK], F16)

        # ---- DMA schedule (consumption order; 3 parallel queues) --------
        # Startup-critical bytes: all of x (re-read per fk from SBUF, so the
        # full 2.4MB gates fk0's last dk chunks) + fk0 weights.  Spread x
        # over all three queues; w2 rides sync/gpsimd AFTER w1/w3 (never on
        # scalar: the ACT queue is FIFO and must stay free for phase-A
        # sigmoids, which release PSUM).
        nc.scalar.dma_start(xsb[:, 0:1, :], xT_d[:, 0:1, :])
        nc.sync.dma_start(w1sb[:, 0, 0:2, :], w1_d[:, 0, 0:2, :])
        nc.gpsimd.dma_start(w3sb[:, 0, 0:2, :], w3_d[:, 0, 0:2, :])
        nc.scalar.dma_start(xsb[:, 1:4, :], xT_d[:, 1:4, :])
        nc.sync.dma_start(w1sb[:, 0, 2:4, :], w1_d[:, 0, 2:4, :])
        nc.gpsimd.dma_start(w3sb[:, 0, 2:4, :], w3_d[:, 0, 2:4, :])
        nc.sync.dma_start(w1sb[:, 0, 4:9, :], w1_d[:, 0, 4:9, :])
        nc.gpsimd.dma_start(w3sb[:, 0, 4:9, :], w3_d[:, 0, 4:9, :])
        nc.sync.dma_start(w1sb[:, 0, 9:KD, :], w1_d[:, 0, 9:KD, :])
        nc.gpsimd.dma_start(w3sb[:, 0, 9:KD, :], w3_d[:, 0, 9:KD, :])
        nc.scalar.dma_start(xsb[:, 4:6, :], xT_d[:, 4:6, :])
        nc.sync.dma_start(xsb[:, 6:8, :], xT_d[:, 6:8, :])
        nc.sync.dma_start(xsb[:, 8:10, :], xT_d[:, 8:10, :])
        nc.gpsimd.dma_start(xsb[:, 10:13, :], xT_d[:, 10:13, :])
        nc.gpsimd.dma_start(xsb[:, 13:KD, :], xT_d[:, 13:KD, :])
        for fk in range(1, KF):
            nc.sync.dma_start(w1sb[:, fk], w1_d[:, fk])
            nc.gpsimd.dma_start(w3sb[:, fk], w3_d[:, fk])
        nc.sync.dma_start(w2sb[:, 0:8], w2_d[:, 0:8])
        nc.gpsimd.dma_start(w2sb[:, 8:KD], w2_d[:, 8:KD])

        # HAM warm-up: keep the PE busy through the prologue DMA wait so
        # the clock gate is at 8/8 when the first real matmul's data lands.
        PSW = 1024 if NTOK > 512 else 512  # psum tile: 2 banks if tail chunk
        ps_w = psp.tile([P, PSW], F32, tag="pa", name="warm_ps")
        for _ in range(5):
            nc.tensor.matmul(ps_w[:, 0:512], warm[:, 0:P], warm[:], start=True, stop=True)

        # ================= phase A: h = silu(x@w1^T) * (x@w3^T) ===========
        # One [P, PSW] psum tile per h1/h3: the 512-wide chunk lands in the
        # first bank, the tail in the second, so the epilogue runs as single
        # wide ops over [0:NTOK] (fewer instructions + semaphore edges).
        for fk in range(KF):
            p1 = psp.tile([P, PSW], F32, tag="pa", name=f"p1_{fk}")
            p3 = psp.tile([P, PSW], F32, tag="pc", name=f"p3_{fk}")
            for dk in range(KD):
                if fk == 0 and dk in (4, 6):
                    # HAM keep-alive during the startup DMA stall: a few
                    # matmuls into the idle warm-up buffer bridge the gap so
                    # the clock gate never re-throttles to 4/8 mid-phase-A.
                    for _ in range(4):
                        nc.tensor.matmul(
                            ps_w[:, 0:512], warm[:, 0:P], warm[:], start=True, stop=True
                        )
                wc1 = w1sb[:, fk, dk, :]
                wc3 = w3sb[:, fk, dk, :]
                st = dk == 0
                sp = dk == KD - 1
                for a, b in CH:
                    nc.tensor.matmul(p1[:, a:b], wc1, xsb[:, dk, a:b], start=st, stop=sp)
                for a, b in CH:
                    nc.tensor.matmul(p3[:, a:b], wc3, xsb[:, dk, a:b], start=st, stop=sp)
            s = spool.tile([P, NTOK], F32, tag="sig", name=f"sig_{fk}")
            nc.scalar.activation(s[:], p1[:, 0:NTOK], ACTF.Sigmoid)
            hs = h[:, fk, :]
            nc.vector.tensor_mul(hs, s[:], p1[:, 0:NTOK])
            nc.vector.tensor_mul(hs, hs, p3[:, 0:NTOK])
            nc.vector.tensor_mul(hsq[:, fk, :], hs, hs)

        # ================= ssq[t] = sum_f h^2 (ones-column matmuls) =======
        sacc = psp.tile([P, PSW], F32, tag="pc", name="sacc")
        for fk in range(KF):
            for a, b in CH:
                nc.tensor.matmul(
                    sacc[0:1, a:b],
                    ones[:],
                    hsq[:, fk, a:b],
                    start=(fk == 0),
                    stop=(fk == KF - 1),
                )
        nc.vector.tensor_copy(ssq_sb[:], sacc[0:1, 0:NTOK])
        nc.scalar.dma_start(ssq_d[:, :], ssq_sb[:])

        # ================= phase C: out[d,t] = sum_fk w2c.T @ h ===========
        # output batches: big batches overlap phase C; tiny last batch so the
        # kernel end is not gated on a large transfer + completion receipt.
        OBATCH = [(0, 5), (5, 10), (10, 13), (13, 15), (15, 16)]
        obt = {
            g0: opool.tile([P, g1 - g0, NTOK], F16, tag=f"ob{g0}", name=f"ob{g0}")
            for g0, g1 in OBATCH
        }
        for gi, (g0, g1) in enumerate(OBATCH):
            ob = obt[g0]
            for dk in range(g0, g1):
                po = psp.tile([P, PSW], F32, tag="pa" if dk % 2 == 0 else "pc", name=f"po_{dk}")
                for fk in range(KF):
                    wc2 = w2sb[:, dk, fk, :]
                    st = fk == 0
                    sp = fk == KF - 1
                    for a, b in CH:
                        nc.tensor.matmul(po[:, a:b], wc2, h[:, fk, a:b], start=st, stop=sp)
                if g1 - g0 == 1 and g1 == KD:
                    # last chunk: split copy+DMA into halves on two queues so
                    # the two completion receipts overlap after the last MM.
                    hn = 288
                    nc.vector.tensor_copy(ob[:, 0, 0:hn], po[:, 0:hn])
                    nc.scalar.dma_start(out_d[:, g0:g1, 0:hn], ob[:, :, 0:hn])
                    nc.vector.tensor_copy(ob[:, 0, hn:NTOK], po[:, hn:NTOK])
                    nc.sync.dma_start(out_d[:, g0:g1, hn:NTOK], ob[:, :, hn:NTOK])
                else:
                    nc.vector.tensor_copy(ob[:, dk - g0, :], po[:, 0:NTOK])
            if not (g1 - g0 == 1 and g1 == KD):
                oq = nc.scalar if gi % 2 == 0 else nc.sync
                oq.dma_start(out_d[:, g0:g1, :], ob[:])

    nc.compile()
    return nc


def _get_program(NTOK: int):
    if NTOK not in _PROGRAM_CACHE:
        _PROGRAM_CACHE[NTOK] = _build_program(NTOK)
    return _PROGRAM_CACHE[NTOK]


def kernel(x, w1, w2, w3, mid_w, num_tokens_per_expert):
    global LAST_RESULTS
    x = np.ascontiguousarray(np.asarray(x, dtype=np.float32))
    w1 = np.asarray(w1, dtype=np.float32)
    w2 = np.asarray(w2, dtype=np.float32)
    w3 = np.asarray(w3, dtype=np.float32)
    mid_w = np.asarray(mid_w, dtype=np.float32)
    counts = np.asarray(num_tokens_per_expert).astype(np.int64)

    T_, D_ = x.shape
    E_, F_, _ = w1.shape
    Ccap = (T_ // E_) * 3 // 2  # reference static capacity (768)
    ends = np.cumsum(counts)
    starts = ends - counts
    eff = np.minimum(np.maximum(counts, 0), Ccap)  # rows actually computed

    NTOK = int(max(64, -(-int(eff.max()) // 64) * 64))  # pad to 64 tokens
    nc = _get_program(NTOK)

    in_maps = []
    for e in range(E_):
        cnt = int(eff[e])
        s = int(starts[e])
        xg = np.zeros((NTOK, D_), np.float32)
        if cnt > 0:
            rows = np.clip(s + np.arange(cnt), 0, T_ - 1)
            xg[:cnt] = x[rows]
        # xT: [P(p), KD, NTOK] with [p, dk, t] = x[t, dk*128+p]
        xT = xg.T.astype(np.float16).reshape(KD, P, NTOK).transpose(1, 0, 2)
        # w1t/w3t: [p, fk, dk, q] = w[fk*128+q, dk*128+p]   (w is [F, D])
        w1t = w1[e].astype(np.float16).reshape(KF, P, KD, P).transpose(3, 0, 2, 1)
        w3t = w3[e].astype(np.float16).reshape(KF, P, KD, P).transpose(3, 0, 2, 1)
        # w2t: [p, dk, fk, q] = w2m[dk*128+q, fk*128+p]     (w2m is [D, F])
        w2m = w2[e] * mid_w[None, :]
        w2t = w2m.astype(np.float16).reshape(KD, P, KF, P).transpose(3, 0, 2, 1)
        in_maps.append(
            {
                "xT": np.ascontiguousarray(xT),
                "w1t": np.ascontiguousarray(w1t),
                "w3t": np.ascontiguousarray(w3t),
                "w2t": np.ascontiguousarray(w2t),
            }
        )

    LAST_RESULTS = _run(nc, in_maps)

    outs = []
    for e in range(E_):
        o = (
            np.asarray(LAST_RESULTS[e]["out"], np.float32)
            .transpose(1, 0, 2)
            .reshape(D_, NTOK)
            .T
        )
        ssq = np.asarray(LAST_RESULTS[e]["ssq"], np.float32).reshape(NTOK)
        rstd = 1.0 / np.sqrt(ssq / F_ + EPS)
        outs.append(o * rstd[:, None])  # [NTOK, D]
    stacked = np.stack(outs, axis=0)  # [E, NTOK, D]

    # scatter back to flat token order, mirroring the reference's clamping
    tok = np.arange(T_)
    eid = np.clip(np.searchsorted(ends, tok, side="right"), 0, E_ - 1)
    pos = tok - starts[eid]
    idx = np.minimum(pos, Ccap - 1)
    valid = (idx >= 0) & (idx < eff[eid])
    idx_safe = np.clip(idx, 0, NTOK - 1)
    result = stacked[eid, idx_safe].astype(np.float32)
    result[~valid] = 0.0
    return result
